# revision 33
# baseline (speedup 1.0000x reference)
import sys

sys.path.insert(0, "/opt/trn_rl_repo")

import numpy as np
import ml_dtypes

import concourse.bass as bass
import concourse.mybir as mybir
from concourse import bass_isa
from concourse.tile import TileContext
from concourse.bass_utils import run_bass_kernel_spmd

dt = mybir.dt
AF = mybir.ActivationFunctionType
OP = mybir.AluOpType

B, T, D, S, R, K = 8, 2048, 512, 2048, 64, 16
P = 128
NT = T // P
NS = S // P
ND = D // P
EPS = 1e-6

_CACHED = {}


_PP = {}


def _split_excess_waits(nc, nopsrc, jp):
    # walrus codegen encodes at most 1 sem wait on PE instructions and 2 on
    # others; move excess waits onto inserted same-engine nops (engines
    # execute their stream in order, so this is equivalent).
    ET = mybir.EngineType
    eng_map = {ET.PE: nc.tensor, ET.DVE: nc.vector, ET.Activation: nc.scalar,
               ET.Pool: nc.gpsimd, ET.SP: nc.sync}

    def make_nop(engine):
        bi = eng_map[engine].nop(nofuse=True, hint="wait_split")
        cur = nc.cur_bb.bb
        lst = cur.instructions
        assert lst[-1] is bi.ins
        cur.instructions = lst[:-1]
        return bi.ins

    # template: a physical 'touch' matmul writing the junk bank
    tmpl = None
    for func in nc.m.functions:
        for blk in func.blocks:
            for inst in blk.instructions:
                if type(inst).__name__ == 'InstMatmult' and inst.outs and                         getattr(inst.outs[0], 'memref', '').startswith('jp'):
                    tmpl = inst
                    break
            if tmpl: break
        if tmpl: break
    assert tmpl is not None, 'no touch matmul template found'

    def make_pe_waiters(waits):
        res = []
        for w in waits:
            m = mybir.InstMatmult(
                name=f'I-waitsplit-{nc.next_id()}',
                ins=list(tmpl.ins), outs=list(tmpl.outs),
                start_tensor_calc=True, stop_tensor_calc=True,
                replication_resolution=tmpl.replication_resolution,
                replication_shift_amnt=tmpl.replication_shift_amnt,
                replication_num_rows=tmpl.replication_num_rows,
            )
            m.engine = ET.PE
            m.sync_info = mybir.SyncInfo(on_wait=[w], on_update=[])
            res.append(m)
        return res

    nsplit = 0
    for func in nc.m.functions:
        for blk in func.blocks:
            out = []
            changed = False
            for inst in blk.instructions:
                si = inst.sync_info
                waits = list(si.on_wait) if si is not None else []
                eng = inst.engine
                if len(waits) > 1 and eng in eng_map:
                    keep, excess = waits[:1], waits[1:]
                    splitters = (make_pe_waiters(excess) if eng == ET.PE
                                 else None)
                    if splitters is None:
                        splitters = []
                        for w in excess:
                            nop = make_nop(eng)
                            nop.sync_info = mybir.SyncInfo(on_wait=[w], on_update=[])
                            splitters.append(nop)
                    # insert before the paired Ldweights if present
                    pos = len(out)
                    if (eng == ET.PE and out and getattr(out[-1], 'concise_opcode', lambda: '')()
                            and 'Ldweights' in str(type(out[-1]))):
                        pos = len(out) - 1
                    out[pos:pos] = splitters
                    nsplit += len(excess)
                    inst.sync_info = mybir.SyncInfo(on_wait=keep, on_update=list(si.on_update))
                    changed = True
                out.append(inst)
            if changed:
                blk.instructions = out
    return nsplit


def build_nc():
    nc = bass.Bass()

    x_d = nc.declare_dram_parameter("x", [T, D], dt.float32, isOutput=False)
    xT_d = nc.declare_dram_parameter("xT", [D, T], dt.float32, isOutput=False)
    w_d = nc.declare_dram_parameter("w", [D, S], dt.float32, isOutput=False)
    pa_d = nc.declare_dram_parameter("pa", [D, R], dt.bfloat16, isOutput=False)
    pb_d = nc.declare_dram_parameter("pb", [D, R], dt.bfloat16, isOutput=False)
    val0_d = nc.declare_dram_parameter("val0", [S, D], dt.float32, isOutput=False)
    st0_d = nc.declare_dram_parameter("state0", [P, NS], dt.float32, isOutput=False)
    idf_d = nc.declare_dram_parameter("identf", [P, P], dt.float32, isOutput=False)
    vf_d = nc.dram_tensor("vf_buf", [S, D], dt.float32)
    idb_d = nc.declare_dram_parameter("identb", [P, P], dt.bfloat16, isOutput=False)
    out_d = nc.declare_dram_parameter("out", [S, 1 + D], dt.float32, isOutput=True)

    with TileContext(nc) as tc:
        with (
            tc.tile_pool(name="const", bufs=1) as cpool,
            tc.tile_pool(name="resV", bufs=1) as vpool,
            tc.tile_pool(name="resA", bufs=1) as apool,
            tc.tile_pool(name="scal", bufs=4) as spool,
            tc.tile_pool(name="junk", bufs=2) as jpool,
        ):
            ones_b = cpool.tile([P, 1], dt.bfloat16, name="ones_b")
            nc.vector.memset(ones_b[:, :], 1.0)

            psJ = tc.alloc_tile_pool(name="psJ", bufs=1, space="PSUM")
            jp = psJ.tile([P, 128], dt.float32, name="jp")

            def touch(src2d, cols):
                # PE observes a freshly DMA'd tile via a dummy matmul into the
                # dedicated junk bank (PE-only; carries exactly one wait).
                nc.tensor.matmul(jp[0:1, 0:cols], src2d[:, 0:1], src2d[:, 0:cols],
                                 start=True, stop=True, skip_group_check=True)

            identf = cpool.tile([P, P], dt.float32, name="identf")
            nc.gpsimd.dma_start(out=identf[:, :], in_=idf_d[:, :])
            touch(identf, 128)
            _PP["identf"] = identf
            _PP["jp"] = jp
            identb = cpool.tile([P, P], dt.bfloat16, name="identb")
            nc.gpsimd.dma_start(out=identb[:, :], in_=idb_d[:, :])
            touch(identb, 128)

            w_sb = []
            for dc in range(ND):
                wt = cpool.tile([P, S], dt.float32, tag=f"wv1t_{dc}", name=f"w_{dc}")
                nc.gpsimd.dma_start(out=wt[:, :], in_=w_d[dc * P:(dc + 1) * P, :])
                touch(wt, 128)
                w_sb.append(wt)

            A_t = []
            rv2 = []
            invz2 = []
            vf_l = []
            vb_l = []

            # ============ Phase AB ============
            rvpool = tc.alloc_tile_pool(name="rvp", bufs=1)
            with (
                tc.tile_pool(name="pAB", bufs=1) as pp,
                tc.tile_pool(name="psB", bufs=2, space="PSUM") as psB,
            ):
                sdacc = pp.tile([P, NS], dt.float32, tag="sdacc", bufs=1, name="sdacc")
                nc.vector.memset(sdacc[:, :], 0.0)
                for tt in range(NT):
                    # xT strips, one tile per d-chunk
                    xTt = []
                    for dc in range(ND):
                        xs = pp.tile([P, P], dt.float32, tag=f"xTt_{dc}", bufs=2, name=f"xs{dc}")
                        nc.gpsimd.dma_start(
                            out=xs[:, :],
                            in_=xT_d[dc * P:(dc + 1) * P, tt * P:(tt + 1) * P],
                        )
                        xTt.append(xs)
                        touch(xs, 128)
                    # reconstruct x rows from strips (PE transpose + DVE copy)
                    xt = pp.tile([P, D], dt.float32, tag="xt", bufs=2, name="xt")
                    for dc in range(ND):
                        ptx = psB.tile([P, P], dt.float32, tag="ps", name="ptx")
                        nc.tensor.transpose(ptx[:, :], xTt[dc][:, :], identf[:, :])
                        nc.vector.tensor_copy(xt[:, dc * P:(dc + 1) * P], ptx[:, :])
                    # row norm of x on DVE (keeps xt readers DVE-only)
                    sqx = jpool.tile([P, D], dt.float32, tag="sq", name="sqx")
                    nc.vector.tensor_tensor(sqx[:, :], xt[:, :], xt[:, :], op=OP.mult)
                    ss = spool.tile([P, 1], dt.float32, tag="ss", name="ss")
                    nc.vector.reduce_sum(ss[:, :], sqx[:, :], axis=mybir.AxisListType.X)
                    nrm = spool.tile([P, 1], dt.float32, tag="nrm", name="nrm")
                    nc.scalar.activation(nrm[:, :], ss[:, :], AF.Sqrt)
                    nrme = spool.tile([P, 1], dt.float32, tag="nrme", name="nrme")
                    nc.vector.tensor_scalar_add(nrme[:, :], nrm[:, :], EPS)
                    rn = spool.tile([P, 1], dt.float32, tag="rn", name="rn")
                    nc.vector.reciprocal(rn[:, :], nrme[:, :])
                    # xT strips, one tile per d-chunk
                    xTt = []
                    for dc in range(ND):
                        xs = pp.tile([P, P], dt.float32, tag=f"xTt_{dc}", bufs=2, name=f"xs{dc}")
                        nc.gpsimd.dma_start(
                            out=xs[:, :],
                            in_=xT_d[dc * P:(dc + 1) * P, tt * P:(tt + 1) * P],
                        )
                        xTt.append(xs)
                    touch(xTt[0], 128)
                    # scores
                    sabs = pp.tile([P, S], dt.float32, tag="sabs", bufs=2, name="sabs")
                    sgn = pp.tile([P, S], dt.bfloat16, tag="sgn", bufs=1, name="sgn")
                    rl = pp.tile([P, S], dt.bfloat16, tag="rl", bufs=1, name="rl")
                    for half in range(2):
                        ps = psB.tile([P, S // 2], dt.float32, tag="ps", name="ps")
                        for fc in range(2):
                            fs = fc * 512
                            for dc in range(ND):
                                nc.tensor.matmul(
                                    ps[:, fs:fs + 512],
                                    xTt[dc][:, :],
                                    w_sb[dc][:, half * (S // 2) + fs:half * (S // 2) + fs + 512],
                                    start=(dc == 0), stop=(dc == ND - 1),
                                )
                        h0 = half * (S // 2)
                        nc.scalar.activation(sabs[:, h0:h0 + S // 2], ps[:, :], AF.Abs)
                        nc.scalar.activation(sgn[:, h0:h0 + S // 2], ps[:, :], AF.Sign)
                        nc.scalar.activation(rl[:, h0:h0 + S // 2], ps[:, :], AF.Relu)
                    # softplus(g) = relu(g) + ln(1 + exp(-|g|))
                    en = pp.tile([P, S], dt.bfloat16, tag="en", bufs=1, name="en")
                    nc.scalar.activation(en[:, :], sabs[:, :], AF.Exp, scale=-1.0)
                    nc.scalar.activation(en[:, :], en[:, :], AF.Ln, bias=1.0)
                    nc.vector.tensor_tensor(rl[:, :], rl[:, :], en[:, :], op=OP.add)
                    # top-16 threshold
                    m1 = spool.tile([P, 8], dt.float32, tag="m1", name="m1")
                    nc.vector.max(m1[:, :], sabs[:, :])
                    srep = pp.tile([P, S], dt.float32, tag="srep", bufs=1, name="srep")
                    nc.vector.match_replace(srep[:, :], m1[:, :], sabs[:, :], -1.0)
                    m2 = spool.tile([P, 8], dt.float32, tag="m2", name="m2")
                    nc.vector.max(m2[:, :], srep[:, :])
                    mask = pp.tile([P, S], dt.bfloat16, tag="mask", bufs=1, name="mask")
                    nc.vector.tensor_scalar(mask[:, :], sabs[:, :], m2[:, 7:8], None, op0=OP.is_ge)
                    expv = pp.tile([P, S], dt.bfloat16, tag="expv", bufs=1, name="expv")
                    nc.scalar.activation(expv[:, :], sabs[:, :], AF.Exp)
                    me1 = pp.tile([P, S], dt.bfloat16, tag="me1", bufs=1, name="me1")
                    nc.vector.tensor_tensor(me1[:, :], mask[:, :], expv[:, :], op=OP.mult)
                    at = apool.tile([P, S], dt.bfloat16, tag=f"a_{tt}", name="at")
                    nc.vector.tensor_tensor(at[:, :], me1[:, :], sgn[:, :], op=OP.mult)
                    A_t.append(at)
                    z1 = spool.tile([P, 1], dt.float32, tag="z1", name="z1")
                    nc.vector.reduce_sum(z1[:, :], me1[:, :], axis=mybir.AxisListType.X)
                    iz = spool.tile([P, 1], dt.float32, tag="iz", name="iz")
                    nc.vector.reciprocal(iz[:, :], z1[:, :])
                    cc = spool.tile([P, 1], dt.float32, tag="cc", name="cc")
                    nc.vector.tensor_tensor(cc[:, :], rn[:, :], iz[:, :], op=OP.mult)
                    rv = rvpool.tile([P, D], dt.bfloat16, tag=f"rv_{tt}", name="rv")
                    nc.vector.tensor_scalar(rv[:, :], xt[:, :], cc[:, :], None, op0=OP.mult)
                    rv2.append(rv)
                    # masked softplus -> per-tile column sums -> sdacc
                    msp = pp.tile([P, S], dt.bfloat16, tag="msp", bufs=1, name="msp")
                    nc.vector.tensor_tensor(msp[:, :], mask[:, :], rl[:, :], op=OP.mult)
                    psd = psB.tile([P, NS], dt.float32, tag="psd", bufs=2, name="psd")
                    for c in range(NS):
                        nc.tensor.matmul(
                            psd[:, c:c + 1],
                            msp[:, c * P:(c + 1) * P],
                            ones_b[:, :],
                            start=True, stop=True,
                        )
                    nc.vector.tensor_tensor(sdacc[:, :], psd[:, :], sdacc[:, :], op=OP.add)

                # ---- state path ----
                st0 = cpool.tile([P, NS], dt.float32, name="st0")
                nc.gpsimd.dma_start(out=st0[:, :], in_=st0_d[:, :])
                stf = cpool.tile([P, NS], dt.float32, name="stf")
                nc.vector.tensor_tensor(stf[:, :], sdacc[:, :], st0[:, :], op=OP.add)
                sta = cpool.tile([P, NS], dt.float32, name="sta")
                nc.scalar.activation(sta[:, :], stf[:, :], AF.Abs)
                # |stf| <= ~40 so exp stays in f32 range; skip the max shift.
                sexp = cpool.tile([P, NS], dt.float32, name="sexp")
                szp = cpool.tile([P, 1], dt.float32, name="szp")
                nc.scalar.activation(sexp[:, :], sta[:, :], AF.Exp, accum_out=szp[:, :])
                ones_m = cpool.tile([P, P], dt.float32, name="ones_m")
                nc.vector.memset(ones_m[:, :], 1.0)
                pzb = psB.tile([P, 1], dt.float32, tag="psd", name="pzb")
                nc.tensor.matmul(pzb[:, :], ones_m[:, :], szp[:, :], start=True, stop=True)
                gz = cpool.tile([P, 1], dt.float32, name="gz")
                nc.vector.tensor_copy(gz[:, :], pzb[:, :])
                giz = cpool.tile([P, 1], dt.float32, name="giz")
                nc.vector.reciprocal(giz[:, :], gz[:, :])
                sgn_s = cpool.tile([P, NS], dt.float32, name="sgn_s")
                nc.scalar.activation(sgn_s[:, :], stf[:, :], AF.Sign)
                st2 = cpool.tile([P, NS], dt.float32, name="st2")
                nc.vector.tensor_tensor(st2[:, :], sexp[:, :], sgn_s[:, :], op=OP.mult)
                sstate = cpool.tile([P, NS], dt.float32, name="sstate")
                nc.vector.tensor_scalar(sstate[:, :], st2[:, :], giz[:, :], None, op0=OP.mult)

            # ============ Phase C: scatter matmul -> val1, v1T via PE transpose ============
            v1T = []
            for dc in range(ND):
                vt = cpool.tile([P, S], dt.bfloat16, tag=f"wv1t_{dc}", name=f"v1T_{dc}")
                v1T.append(vt)
            with (
                tc.tile_pool(name="pC", bufs=2) as pC,
                tc.tile_pool(name="psC", bufs=2, space="PSUM") as psC,
            ):
                for sb in range(NS):
                    pvd = psC.tile([P, D], dt.float32, tag="pvd", name="pvd")
                    nc.vector.memset(pvd[:, :], 0.0)
                    for tt in range(NT):
                        nc.tensor.matmul(
                            pvd[:, :],
                            A_t[tt][:, sb * P:(sb + 1) * P],
                            rv2[tt][:, :],
                            start=(tt == 0), stop=(tt == NT - 1),
                        )
                    v0 = pC.tile([P, D], dt.float32, tag="v0", name="v0")
                    nc.gpsimd.dma_start(out=v0[:, :], in_=val0_d[sb * P:(sb + 1) * P, :])
                    r1 = pC.tile([P, D], dt.float32, tag="r1", name="r1")
                    nc.vector.tensor_tensor(r1[:, :], pvd[:, :], v0[:, :], op=OP.add)
                    sq1 = jpool.tile([P, D], dt.float32, tag="sq", name="sq1")
                    nc.vector.tensor_tensor(sq1[:, :], r1[:, :], r1[:, :], op=OP.mult)
                    ss1 = spool.tile([P, 1], dt.float32, tag="ss1", name="ss1")
                    nc.vector.reduce_sum(ss1[:, :], sq1[:, :], axis=mybir.AxisListType.X)
                    nr1 = spool.tile([P, 1], dt.float32, tag="nr1", name="nr1")
                    nc.scalar.activation(nr1[:, :], ss1[:, :], AF.Sqrt)
                    nr1e = spool.tile([P, 1], dt.float32, tag="nr1e", name="nr1e")
                    nc.vector.tensor_scalar_add(nr1e[:, :], nr1[:, :], EPS)
                    rn1 = spool.tile([P, 1], dt.float32, tag="rn1", name="rn1")
                    nc.vector.reciprocal(rn1[:, :], nr1e[:, :])
                    vf = pC.tile([P, D], dt.float32, tag="vf", name="vf")
                    nc.vector.tensor_scalar(vf[:, :], r1[:, :], rn1[:, :], None, op0=OP.mult)
                    nc.gpsimd.dma_start(out=vf_d[sb * P:(sb + 1) * P, :], in_=vf[:, :])
                    vb = vpool.tile([P, D], dt.bfloat16, tag=f"vb_{sb}", name="vb")
                    nc.vector.tensor_copy(vb[:, :], vf[:, :])
                    vb_l.append(vb)
                    for dc in range(ND):
                        ptv = psC.tile([P, P], dt.float32, tag="ptv", name="ptv")
                        nc.tensor.transpose(ptv[:, :], vf[:, dc * P:(dc + 1) * P], identf[:, :])
                        nc.vector.tensor_copy(v1T[dc][:, sb * P:(sb + 1) * P], ptv[:, :])
            rvpool.release()

            # ============ Phase D: qT, kT ============
            qT_sb = cpool.tile([R, S], dt.bfloat16, name="qT_sb")
            kT_sb = cpool.tile([R, S], dt.bfloat16, name="kT_sb")
            with (
                tc.tile_pool(name="pD", bufs=1) as pD,
                tc.tile_pool(name="psQ", bufs=1, space="PSUM") as psQ,
            ):
                pa_sb = pD.tile([P, ND, R], dt.bfloat16, tag="pa", name="pa_sb")
                nc.gpsimd.dma_start(out=pa_sb[:, :, :], in_=pa_d[:, :].rearrange("(n p) r -> p n r", p=P))
                touch(pa_sb[:, 0, :], R)
                pb_sb = pD.tile([P, ND, R], dt.bfloat16, tag="pb", name="pb_sb")
                nc.gpsimd.dma_start(out=pb_sb[:, :, :], in_=pb_d[:, :].rearrange("(n p) r -> p n r", p=P))
                touch(pb_sb[:, 0, :], R)
                for h in range(2):
                    pq = psQ.tile([R, S // 2], dt.float32, tag="pq", bufs=1, name="pq")
                    pk = psQ.tile([R, S // 2], dt.float32, tag="pk", bufs=1, name="pk")
                    for fc in range(2):
                        gfc = h * 2 + fc
                        for dc in range(ND):
                            nc.tensor.matmul(
                                pq[:, fc * 512:(fc + 1) * 512],
                                pa_sb[:, dc, :],
                                v1T[dc][:, gfc * 512:(gfc + 1) * 512],
                                start=(dc == 0), stop=(dc == ND - 1),
                            )
                            nc.tensor.matmul(
                                pk[:, fc * 512:(fc + 1) * 512],
                                pb_sb[:, dc, :],
                                v1T[dc][:, gfc * 512:(gfc + 1) * 512],
                                start=(dc == 0), stop=(dc == ND - 1),
                            )
                    nc.vector.tensor_copy(qT_sb[:, h * (S // 2):(h + 1) * (S // 2)], pq[:, :])
                    nc.vector.tensor_copy(kT_sb[:, h * (S // 2):(h + 1) * (S // 2)], pk[:, :])

            # ============ Phase E: pairwise p, topk mask, ME2 (A slots) ============
            ME2 = []
            with (
                tc.tile_pool(name="pE", bufs=1) as pE,
                tc.tile_pool(name="psE", bufs=2, space="PSUM") as psE,
            ):
                for si in range(NS):
                    pabs = pE.tile([P, S], dt.float32, tag="pabs", bufs=1, name="pabs")
                    psgn = pE.tile([P, S], dt.bfloat16, tag="psgn", bufs=1, name="psgn")
                    for half in range(2):
                        ppp = psE.tile([P, S // 2], dt.float32, tag="pp", name="ppp")
                        for fc in range(2):
                            fs = half * (S // 2) + fc * 512
                            nc.tensor.matmul(
                                ppp[:, fc * 512:(fc + 1) * 512],
                                qT_sb[:, si * P:(si + 1) * P],
                                kT_sb[:, fs:fs + 512],
                                start=True, stop=True,
                            )
                        h0 = half * (S // 2)
                        nc.scalar.activation(pabs[:, h0:h0 + S // 2], ppp[:, :], AF.Abs)
                        nc.scalar.activation(psgn[:, h0:h0 + S // 2], ppp[:, :], AF.Sign)
                    m1p = spool.tile([P, 8], dt.float32, tag="m1p", name="m1p")
                    nc.vector.max(m1p[:, :], pabs[:, :])
                    prep = pE.tile([P, S], dt.float32, tag="prep", bufs=1, name="prep")
                    nc.vector.match_replace(prep[:, :], m1p[:, :], pabs[:, :], -1.0)
                    m2p = spool.tile([P, 8], dt.float32, tag="m2p", name="m2p")
                    nc.vector.max(m2p[:, :], prep[:, :])
                    mask2 = pE.tile([P, S], dt.bfloat16, tag="mask2", bufs=1, name="mask2")
                    nc.vector.tensor_scalar(mask2[:, :], pabs[:, :], m2p[:, 7:8], None, op0=OP.is_ge)
                    expv2 = pE.tile([P, S], dt.bfloat16, tag="expv2", bufs=1, name="expv2")
                    nc.scalar.activation(expv2[:, :], pabs[:, :], AF.Exp)
                    me2u = pE.tile([P, S], dt.bfloat16, tag="me2u", bufs=1, name="me2u")
                    nc.vector.tensor_tensor(me2u[:, :], mask2[:, :], expv2[:, :], op=OP.mult)
                    met = apool.tile([P, S], dt.bfloat16, tag=f"a_{si}", name="met")
                    nc.vector.tensor_tensor(met[:, :], me2u[:, :], psgn[:, :], op=OP.mult)
                    ME2.append(met)
                    z2 = spool.tile([P, 1], dt.float32, tag="z2", name="z2")
                    nc.vector.reduce_sum(z2[:, :], me2u[:, :], axis=mybir.AxisListType.X)
                    iz2 = spool.tile([P, 1], dt.float32, tag=f"iz2_{si}", bufs=1, name="iz2")
                    nc.vector.reciprocal(iz2[:, :], z2[:, :])
                    invz2.append(iz2)

            # ============ Phase F: MET strips via PE transpose, delta matmul, out ============
            with (
                tc.tile_pool(name="pF", bufs=2) as pF,
                tc.tile_pool(name="psF", bufs=2, space="PSUM") as psF,
                tc.tile_pool(name="psDlt", bufs=1, space="PSUM") as psDlt,
            ):
                GRP = [list(range(0, 5)), list(range(5, 10)), list(range(10, 15)), list(range(15, 16))]
                for grp in GRP:
                    pdelta = []
                    for i, sb in enumerate(grp):
                        pd = psDlt.tile([P, D], dt.float32, tag=f"pd_{i}", name=f"pd{i}")
                        nc.vector.memset(pd[:, :], 0.0)
                        pdelta.append(pd)
                    for jt in range(NS):
                        strip = pF.tile([P, len(GRP[0]) * P], dt.bfloat16, tag="strip", name="strip")
                        for i, sb in enumerate(grp):
                            ptm = psF.tile([P, P], dt.bfloat16, tag="ptm", name="ptm")
                            nc.tensor.transpose(ptm[:, :], ME2[sb][:, jt * P:(jt + 1) * P], identb[:, :])
                            nc.vector.tensor_copy(strip[:, i * P:(i + 1) * P], ptm[:, :])
                        for i, sb in enumerate(grp):
                            nc.tensor.matmul(
                                pdelta[i][:, :],
                                strip[:, i * P:(i + 1) * P],
                                vb_l[jt][:, :],
                                start=(jt == 0), stop=(jt == NS - 1),
                            )
                    for i, sb in enumerate(grp):
                        dd = pF.tile([P, D], dt.float32, tag="dd", name="dd")
                        nc.vector.tensor_scalar(dd[:, :], pdelta[i][:, :], invz2[sb][:, :], None, op0=OP.mult)
                        vfl = pF.tile([P, D], dt.float32, tag="vfl", name="vfl")
                        nc.gpsimd.dma_start(out=vfl[:, :], in_=vf_d[sb * P:(sb + 1) * P, :])
                        r2 = pF.tile([P, D], dt.float32, tag="r2", name="r2")
                        nc.vector.tensor_tensor(r2[:, :], dd[:, :], vfl[:, :], op=OP.add)
                        sq2 = jpool.tile([P, D], dt.float32, tag="sq", name="sq2")
                        nc.vector.tensor_tensor(sq2[:, :], r2[:, :], r2[:, :], op=OP.mult)
                        ss2 = spool.tile([P, 1], dt.float32, tag="ss2", name="ss2")
                        nc.vector.reduce_sum(ss2[:, :], sq2[:, :], axis=mybir.AxisListType.X)
                        nr2 = spool.tile([P, 1], dt.float32, tag="nr2", name="nr2")
                        nc.scalar.activation(nr2[:, :], ss2[:, :], AF.Sqrt)
                        nr2e = spool.tile([P, 1], dt.float32, tag="nr2e", name="nr2e")
                        nc.vector.tensor_scalar_add(nr2e[:, :], nr2[:, :], EPS)
                        rn2 = spool.tile([P, 1], dt.float32, tag="rn2", name="rn2")
                        nc.vector.reciprocal(rn2[:, :], nr2e[:, :])
                        v2 = pF.tile([P, D], dt.float32, tag="v2", name="v2")
                        nc.vector.tensor_scalar(v2[:, :], r2[:, :], rn2[:, :], None, op0=OP.mult)
                        nc.gpsimd.dma_start(out=out_d[sb * P:(sb + 1) * P, 1:], in_=v2[:, :])
                        nc.gpsimd.dma_start(out=out_d[sb * P:(sb + 1) * P, 0:1], in_=sstate[:, sb:sb + 1])
            psJ.release()

    _split_excess_waits(nc, _PP["identf"], _PP["jp"])
    return nc


def kernel(x, init_state, init_val, route_a, route_b, pair_a, pair_b):
    x = np.asarray(x, dtype=np.float32)
    init_state = np.asarray(init_state, dtype=np.float32)
    init_val = np.asarray(init_val, dtype=np.float32)
    route_a = np.asarray(route_a, dtype=np.float32)
    route_b = np.asarray(route_b, dtype=np.float32)
    pair_a = np.asarray(pair_a, dtype=np.float32)
    pair_b = np.asarray(pair_b, dtype=np.float32)

    W = route_a @ route_b
    e = np.exp(np.abs(init_state) - np.abs(init_state).max())
    state0 = (np.sign(init_state) * (e / e.sum())).astype(np.float32)
    state0 = np.ascontiguousarray(state0.reshape(NS, P).T)
    val0 = (init_val / (np.linalg.norm(init_val, axis=-1, keepdims=True) + EPS)).astype(np.float32)

    if "nc" not in _CACHED:
        _CACHED["nc"] = build_nc()
    nc = _CACHED["nc"]

    in_maps = []
    for b in range(B):
        in_maps.append({
            "x": np.ascontiguousarray(x[b]),
            "xT": np.ascontiguousarray(x[b].T),
            "w": W,
            "pa": pair_a.astype(ml_dtypes.bfloat16),
            "pb": pair_b.astype(ml_dtypes.bfloat16),
            "val0": val0, "state0": state0,
            "identf": np.eye(P, dtype=np.float32),
            "identb": np.eye(P).astype(ml_dtypes.bfloat16),
        })
    import os
    trace = bool(os.environ.get("KERNEL_TRACE"))
    res = run_bass_kernel_spmd(nc, in_maps, list(range(B)), trace=trace,
                               tmpdir=os.environ.get("KERNEL_TRACE_DIR") or None)
    if res.exec_time_ns is not None:
        print(f"HW exec time: {res.exec_time_ns} ns")
    out = np.stack([np.asarray(res.results[b]["out"]) for b in range(B)], axis=0)
    return out.astype(np.float32)
